# revision 1
# baseline (speedup 1.0000x reference)
"""Spiking transformer block (SpikingRetention + spiking MLP) on 8 Trainium2 cores.

Data-parallel over B=8 (one batch element per NeuronCore), weights replicated.

Layouts: activations are feature-major ("transposed", [C part, N free]) so the
folded BatchNorm is a per-partition ACT affine; v is computed directly in
natural layout ([N part, C free]) with its bias folded into the matmul as a
rank-1 ones term. LIF membranes are kept 2^t-scaled (A_t = 2^t u_t) so each
step is add / compare / masked-reset with exact power-of-2 constants.

Matmul precision: dense matmuls run in float32r (single-pass PE mode, ~1e-4
relative error on HW). Scores and spike-weighted matmuls are fp16 (exact
binary spikes).

Scheduling: the whole kernel is a t-outer wavefront (all 40 LIF chains
advance one step per wave); retention is software-pipelined one wave behind
qkv. Within a wave the LIF ops are emitted in op-type layers (all extracts,
all adds, all spikes, all carry updates) so the in-order engine sequencers
never head-of-line block on a single chain.
"""

from contextlib import ExitStack

import numpy as np

import concourse.bacc as bacc
import concourse.tile as tile
from concourse import mybir
from concourse.bass_utils import run_bass_kernel_spmd

f32 = mybir.dt.float32
f32r = mybir.dt.float32r
fp16 = mybir.dt.float16
Alu = mybir.AluOpType
Act = mybir.ActivationFunctionType

T, B, N, C = 4, 8, 512, 512
HID = 2048
H = 8
EPS = 1e-5
NT = N // 128
CT = C // 128
HT = HID // 128

_CACHE = {}


def _lif_extract(nc, pool, ps, t, bias_ap):
    """A0 = 2^(t-1)*psum + bias (per-partition bias AP or None). Frees psum."""
    sc = float(2.0 ** (t - 1))
    A = pool.tile([128, 512], f32, name="lifzb", tag="lifzb", bufs=12)
    if bias_ap is None:
        nc.scalar.activation(A[:], ps[:], Act.Copy, bias=0.0, scale=sc)
    else:
        nc.scalar.activation(A[:], ps[:], Act.Identity, bias=bias_ap, scale=sc)
    return A


def _lif_layers(nc, work, specs, thr_tiles):
    """Finish LIF steps for a batch of chains, op-type layered.
    spec: (A, t, theta, carry_in, spike_tile, cpool, carry_tag, aadd_eng,
    spike_act). The carry add happens in place over A."""
    for (A, t, th, cin, st, cp, ctag, ae, sa) in specs:
        if cin is not None:
            ae.tensor_tensor(A[:], cin[:], A[:], Alu.add)
    for (A, t, th, cin, st, cp, ctag, ae, sa) in specs:
        if sa:
            thr = float(th * (2.0 ** t))
            rl = work.tile([128, 512], f32, name="lifrl", tag="lifrl", bufs=4)
            nc.scalar.activation(rl[:], A[:], Act.Relu, bias=thr_tiles[thr][:])
            nc.scalar.activation(st[:], rl[:], Act.Sign)
    for (A, t, th, cin, st, cp, ctag, ae, sa) in specs:
        if not sa:
            thr = float(th * (2.0 ** t))
            nc.vector.tensor_scalar(st[:], A[:], thr, None, Alu.is_ge)
    out = []
    for (A, t, th, cin, st, cp, ctag, ae, sa) in specs:
        if ctag is not None:
            thr = float(th * (2.0 ** t))
            Cn = cp.tile([128, 512], f32, name="lifC", tag=ctag, bufs=1)
            nc.vector.scalar_tensor_tensor(Cn[:], A[:], thr, A[:], Alu.is_lt,
                                           Alu.mult)
            out.append(Cn)
        else:
            out.append(None)
    return out


def _build():
    nc = bacc.Bacc("TRN2", target_bir_lowering=False, debug=False)

    xb = nc.declare_dram_parameter("xb", [T, CT, 128, N], f32r, isOutput=False)
    wqkv_e = nc.declare_dram_parameter("wqkv", [128, 3 * CT, C], f32r,
                                       isOutput=False)
    w_in = {}
    for nm, ci, co in [("pw", C, C), ("w1", C, HID), ("w2", HID, C)]:
        dt_w = fp16 if nm in ("pw", "w2") else f32r
        w_in[nm] = nc.declare_dram_parameter(nm, [128, ci // 128, co], dt_w,
                                             isOutput=False)
    bias_e = nc.declare_dram_parameter("biases", [128, T, 32], f32,
                                       isOutput=False)
    vb_e = nc.declare_dram_parameter("vrow", [1, C + 128], f32r, isOutput=False)
    dmat_e = nc.declare_dram_parameter("dmat", [H, 128, NT, N], fp16, isOutput=False)
    out_e = nc.declare_dram_parameter("out", [T, CT, 128, N], fp16, isOutput=True)

    DVE = nc.vector
    POOL = nc.gpsimd

    with tile.TileContext(nc) as tc, ExitStack() as ctx:
        pers = ctx.enter_context(tc.tile_pool(name="pers", bufs=1))
        work = ctx.enter_context(tc.tile_pool(name="work", bufs=1))

        thr_tiles = {}
        for tv in (1.0, 2.0, 4.0, 8.0, 16.0):
            tt_ = pers.tile([128, 1], f32, name=f"nthr{int(tv)}")
            nc.vector.memset(tt_[:], -tv)
            thr_tiles[tv] = tt_
        bias_sb = {}

        os_ = {}
        with tc.tile_pool(name="pw_pool", bufs=1) as pw_pool, \
             tc.tile_pool(name="spk_o_pool", bufs=1) as spk_o_pool:
            pwt = pw_pool.tile([128, CT, C], fp16, name="w_pw")

            # ---- stage A (t-outer wavefront): qkv / v-natural / scores*D /
            # retention, with retention pipelined one wave behind qkv.
            with tc.tile_pool(name="wqkv_pool", bufs=1) as wqkv_pool, \
                 tc.tile_pool(name="xa_pool", bufs=1) as xa_pool, \
                 tc.tile_pool(name="spk_pool", bufs=1) as spk_pool, \
                 tc.tile_pool(name="carry_pool", bufs=1) as carry_pool, \
                 tc.tile_pool(name="dm_pool", bufs=1) as dm_pool, \
                 tc.tile_pool(name="spool", bufs=1) as spool, \
                 tc.tile_pool(name="psA", bufs=1, space="PSUM") as psA:
                # startup loads all on the Pool SWDGE queue: hoisted preps keep
                # their relative emission order, so the exclusive DMA device
                # serves them in this priority order (x1, qw, kw, vw, pw) with
                # the later-emitted prefetch/dm preps queued behind. Tiny bias
                # rows go on the Act hwdge queue (sub-us transfers).
                xwt = xa_pool.tile([128, CT, N], f32r, name="xT", tag="xT", bufs=2)
                wqkv_t = wqkv_pool.tile([128, 3 * CT, C], f32r, name="w_qkv")
                ball = pers.tile([128, T, 32], f32, name="ball")
                vrow = pers.tile([1, C + 128], f32r, name="vrow")
                POOL.dma_start(xwt[:], xb[0].rearrange("ct p n -> p ct n"))
                xw = {ct: xwt[:, ct, :] for ct in range(CT)}
                for i in range(3):
                    POOL.dma_start(wqkv_t[:, i * CT:(i + 1) * CT, :],
                                   wqkv_e[:, i * CT:(i + 1) * CT, :])
                nc.scalar.dma_start(ball[:], bias_e[:, :, :])
                nc.scalar.dma_start(vrow[:], vb_e[:, :])
                wq = {nm: wqkv_t[:, i * CT:(i + 1) * CT, :]
                      for i, nm in enumerate(("qw", "kw", "vw"))}
                bias_sb = {"qb": ball[:, :, 0:4], "kb": ball[:, :, 4:8],
                           "pb": ball[:, :, 8:12], "b2": ball[:, :, 12:16],
                           "b1": ball[:, :, 16:32]}
                vbrow = vrow[:, 0:C]
                ones128 = vrow[:, C:C + 128]
                dmt = dm_pool.tile([128, H, NT, N], fp16, name="dmt")
                dms = [dmt[:, h] for h in range(H)]
                POOL.dma_start(pwt[:], w_in["pw"][:, :, :])

                cq = {}
                co_ = {}

                def ret_scores(hp, qs_p, ks_p):
                    h0, h1 = 2 * hp, 2 * hp + 1
                    spairs = []
                    for mtp in range(NT // 2):
                        ps0 = psA.tile([128, 2, N], f32, name="ps_s0",
                                       tag="ps_s0", bufs=1)
                        ps1 = psA.tile([128, 2, N], f32, name="ps_s1",
                                       tag="ps_s1", bufs=1)
                        for j in range(2):
                            mt = 2 * mtp + j
                            nc.tensor.matmul(
                                ps0[:, j, :],
                                ks_p[hp][0:64, mt * 128:(mt + 1) * 128],
                                qs_p[hp][0:64, :], start=True, stop=True)
                            nc.tensor.matmul(
                                ps1[:, j, :],
                                ks_p[hp][64:128, mt * 128:(mt + 1) * 128],
                                qs_p[hp][64:128, :], start=True, stop=True)
                        s0 = spool.tile([128, 2, N], fp16, name="sd0",
                                        tag="sd0", bufs=3)
                        DVE.tensor_tensor(
                            s0[:], ps0[:],
                            dms[h0][:, 2 * mtp:2 * mtp + 2, :], Alu.mult)
                        s1 = spool.tile([128, 2, N], fp16, name="sd1",
                                        tag="sd1", bufs=3)
                        DVE.tensor_tensor(
                            s1[:], ps1[:],
                            dms[h1][:, 2 * mtp:2 * mtp + 2, :], Alu.mult)
                        spairs.append((s0, s1))
                    return spairs

                def ret_out(hp, spairs, vn_p, t_r, rspecs):
                    h0, h1 = 2 * hp, 2 * hp + 1
                    pso = psA.tile([128, N], f32, name="ps_o", tag="ps_o", bufs=1)
                    for mt in range(NT):
                        s0, s1 = spairs[mt // 2]
                        nc.tensor.matmul(
                            pso[0:64, :],
                            vn_p[mt][:, h0 * 64:(h0 + 1) * 64],
                            s0[:, mt % 2, :],
                            start=(mt == 0), stop=(mt == NT - 1))
                        nc.tensor.matmul(
                            pso[64:128, :],
                            vn_p[mt][:, h1 * 64:(h1 + 1) * 64],
                            s1[:, mt % 2, :],
                            start=(mt == 0), stop=(mt == NT - 1))
                    A = _lif_extract(nc, work, pso, t_r, None)
                    st = spk_o_pool.tile([128, N], fp16, name="spk_os",
                                         tag="spk_os", bufs=16)
                    rspecs.append((A, t_r, 0.5, co_.get(hp), st, spool,
                                   f"c_o{hp}" if t_r < T else None, POOL, True))
                    os_[t_r - 1, hp] = st

                prev = None
                xw_next = None
                for t in range(1, T + 1):
                    if t > 1:  # x tiles were prefetched during the prior wave
                        xw = xw_next
                    qs_c = {}
                    ks_c = {}
                    vn_c = {}
                    specs = []
                    keys = []

                    def emit_qk(nm, bnm, dst, ot, t=t):
                        ps = psA.tile([128, N], f32, name="psq", tag="psq", bufs=3)
                        for kt in range(CT):
                            nc.tensor.matmul(ps[:],
                                             wq[nm][:, kt, ot * 128:(ot + 1) * 128],
                                             xw[kt],
                                             start=(kt == 0), stop=(kt == CT - 1))
                        A = _lif_extract(nc, work, ps, t,
                                         bias_sb[bnm][:, t - 1, ot:ot + 1])
                        st = spk_pool.tile([128, N], fp16, name=f"spk_{nm}",
                                           tag=f"spk_{nm}", bufs=8)
                        dst[ot] = st
                        specs.append((A, t, 1.0, cq.get((nm, ot)), st, carry_pool,
                                      f"c_{nm}{ot}" if t < T else None, POOL, True))
                        keys.append((nm, ot))

                    def emit_v(nt, t=t):
                        ps = psA.tile([128, C], f32, name="psv", tag="psq", bufs=3)
                        for kt in range(CT):
                            nc.tensor.matmul(ps[:],
                                             xw[kt][:, nt * 128:(nt + 1) * 128],
                                             wq["vw"][:, kt, :],
                                             start=(kt == 0), stop=False)
                        nc.tensor.matmul(ps[:], ones128, vbrow,
                                         start=False, stop=True)
                        A = _lif_extract(nc, work, ps, t, None)
                        st = spk_pool.tile([128, C], fp16, name="vn", tag="vn",
                                           bufs=8)
                        vn_c[nt] = st
                        specs.append((A, t, 1.0, cq.get(("vw", nt)), st,
                                      carry_pool,
                                      f"c_vw{nt}" if t < T else None, POOL, False))
                        keys.append(("vw", nt))

                    groups = [lambda ot=ot: emit_qk("qw", "qb", qs_c, ot)
                              for ot in range(CT)]
                    groups += [lambda ot=ot: emit_qk("kw", "kb", ks_c, ot)
                               for ot in range(CT)]
                    groups += [lambda nt=nt: emit_v(nt) for nt in range(NT)]

                    rspecs = []
                    if prev is not None:
                        # interleave retention (wave t-1) head-pair blocks with
                        # this wave's qkv groups so the retention-out matmuls
                        # never wait on the decay-multiply
                        qs_p, ks_p, vn_p = prev
                        sp = {}
                        order = [0, 1, 2, ("s", 0), 3, 4, ("o", 0), ("s", 1),
                                 5, 6, ("o", 1), ("s", 2), 7, 8, ("o", 2),
                                 ("s", 3), 9, 10, ("o", 3), 11]
                        for item in order:
                            if isinstance(item, int):
                                groups[item]()
                            elif item[0] == "s":
                                sp[item[1]] = ret_scores(item[1], qs_p, ks_p)
                            else:
                                ret_out(item[1], sp[item[1]], vn_p, t - 1, rspecs)
                    else:
                        for g in groups:
                            g()
                    carries = _lif_layers(nc, work, specs, thr_tiles)
                    for k, cr in zip(keys, carries):
                        cq[k] = cr
                    rcarr = _lif_layers(nc, work, rspecs, thr_tiles)
                    for hp, cr in enumerate(rcarr):
                        co_[hp] = cr
                    if t < T:  # prefetch next wave's x on the SWDGE queue
                        xwt = xa_pool.tile([128, CT, N], f32r, name="xT",
                                           tag="xT", bufs=2)
                        POOL.dma_start(xwt[:],
                                       xb[t].rearrange("ct p n -> p ct n"))
                        xw_next = {ct: xwt[:, ct, :] for ct in range(CT)}
                    if t == 1:  # decay matrices: requested after wave-2 x
                        for hp in range(4):
                            POOL.dma_start(
                                dmt[:, 2 * hp:2 * hp + 2],
                                dmat_e.rearrange("h p nt n -> p h nt n")
                                [:, 2 * hp:2 * hp + 2])
                    prev = (qs_c, ks_c, vn_c)
                # final retention wave (t = T)
                qs_p, ks_p, vn_p = prev
                rspecs = []
                for hp in range(H // 2):
                    spairs = ret_scores(hp, qs_p, ks_p)
                    ret_out(hp, spairs, vn_p, T, rspecs)
                rcarr = _lif_layers(nc, work, rspecs, thr_tiles)

            # ---- stage B (t-outer): proj + MLP + output
            with tc.tile_pool(name="wmlp_pool", bufs=1) as wmlp_pool, \
                 tc.tile_pool(name="mwork", bufs=1) as mwork, \
                 tc.tile_pool(name="xtin_pool", bufs=1) as xtin_pool, \
                 tc.tile_pool(name="psM", bufs=1, space="PSUM") as psM:
                # w1/w2 loads split into chunks so the per-wave x loads (on the
                # Pool SWDGE queue) can interleave on the shared DMA engines
                w1t = wmlp_pool.tile([128, CT, HID], f32r, name="w_w1")
                w2t = wmlp_pool.tile([128, HT, C], fp16, name="w_w2")
                for kt in range(CT):
                    nc.sync.dma_start(w1t[:, kt], w_in["w1"][:, kt])
                for ktp in range(2):
                    nc.sync.dma_start(w2t[:, 8 * ktp:8 * ktp + 8],
                                      w_in["w2"][:, 8 * ktp:8 * ktp + 8])
                wmlp = {"w1": w1t, "w2": w2t}

                cp = {}
                c1 = {}
                c2 = {}
                x2_all = {}
                ht_all = {}

                def fc2_wave(t):
                    # fc2 of wave t, emitted during wave t+1 (htile ready)
                    htile = ht_all.pop(t)
                    x2 = x2_all.pop(t)
                    last = (t == T)
                    specs = []
                    sts = []
                    for ot in range(CT):
                        ps = psM.tile([128, N], f32, name="psf2", tag="psf2", bufs=2)
                        for kt in range(HT):
                            nc.tensor.matmul(ps[:],
                                             wmlp["w2"][:, kt, ot * 128:(ot + 1) * 128],
                                             htile[kt][:], start=(kt == 0),
                                             stop=(kt == HT - 1))
                        A = _lif_extract(nc, work, ps, t,
                                         bias_sb["b2"][:, t - 1, ot:ot + 1])
                        st = mwork.tile([128, N], f32, name="spk_m", tag="spk_m",
                                        bufs=2)
                        sts.append(st)
                        specs.append((A, t, 1.0, c2.get(ot), st, mwork,
                                      f"c2_{ot}" if t < T else None,
                                      DVE if last else POOL, False))
                    carries = _lif_layers(nc, work, specs, thr_tiles)
                    outb = mwork.tile([128, CT, N], fp16, name="outb", tag="outb",
                                      bufs=1)
                    for ot in range(CT):
                        c2[ot] = carries[ot]
                        DVE.tensor_tensor(outb[:, ot, :], x2[ot], sts[ot][:],
                                           Alu.add)
                        if last:  # drain the store per tile at the kernel tail
                            nc.sync.dma_start(out_e[t - 1, ot], outb[:, ot, :])
                    if not last:
                        nc.scalar.dma_start(
                            out_e[t - 1].rearrange("ct p n -> p ct n"), outb[:])

                for t in range(1, T + 1):
                    xin = xtin_pool.tile([128, CT, N], f32r, name="xtin",
                                         tag="xtin", bufs=1)
                    POOL.dma_start(xin[:], xb[t - 1].rearrange("ct p n -> p ct n"))
                    # proj linear + LIF -> attn spikes; x2 = x + attn (SBUF only)
                    x2 = {}
                    specs = []
                    stps = []
                    for ot in range(CT):
                        ps = psM.tile([128, N], f32, name="psp", tag="psp", bufs=2)
                        for kt in range(CT):
                            nc.tensor.matmul(ps[:], pwt[:, kt, ot * 128:(ot + 1) * 128],
                                             os_[t - 1, kt][:], start=(kt == 0),
                                             stop=(kt == CT - 1))
                        A = _lif_extract(nc, work, ps, t,
                                         bias_sb["pb"][:, t - 1, ot:ot + 1])
                        stp = mwork.tile([128, N], f32, name="spk_p", tag="spk_p",
                                         bufs=2)
                        stps.append(stp)
                        specs.append((A, t, 1.0, cp.get(ot), stp, mwork,
                                      f"cp_{ot}" if t < T else None, POOL, True))
                    carries = _lif_layers(nc, work, specs, thr_tiles)
                    x2b = mwork.tile([128, CT, N], f32r, name="x2t", tag="x2t", bufs=2)
                    for ot in range(CT):
                        cp[ot] = carries[ot]
                        DVE.tensor_tensor(x2b[:, ot, :], xin[:, ot, :],
                                           stps[ot][:], Alu.add)
                        x2[ot] = x2b[:, ot, :]
                    x2_all[t] = x2
                    if t > 1:
                        fc2_wave(t - 1)
                    htile = {}
                    specs = []
                    for ot in range(HT):
                        ps = psM.tile([128, N], f32, name="psf1", tag="psf1", bufs=4)
                        for kt in range(CT):
                            nc.tensor.matmul(ps[:],
                                             wmlp["w1"][:, kt, ot * 128:(ot + 1) * 128],
                                             x2[kt], start=(kt == 0),
                                             stop=(kt == CT - 1))
                        A = _lif_extract(nc, work, ps, t,
                                         bias_sb["b1"][:, t - 1, ot:ot + 1])
                        st = mwork.tile([128, N], fp16, name="spk_h", tag="spk_h",
                                        bufs=HT)
                        htile[ot] = st
                        specs.append((A, t, 1.0, c1.get(ot), st, mwork,
                                      f"c1_{ot}" if t < T else None,
                                      POOL, ot % 4 == 1))
                        if ot % 4 == 3:  # flush in quarters to bound zb usage
                            carries = _lif_layers(nc, work, specs, thr_tiles)
                            for j, cr in enumerate(carries):
                                c1[ot - 3 + j] = cr
                            specs = []
                    ht_all[t] = htile
                fc2_wave(T)

    nc.finalize()
    return nc


def _host_prep(inputs):
    def fold(w, b, bn):
        g, bb, m, v = [bn[i].astype(np.float64) for i in range(4)]
        A = g / np.sqrt(v + EPS)
        W = w.astype(np.float64) * A[:, None]
        bias = (b.astype(np.float64) - m) * A + bb
        return W, bias

    def bias_layout(bias):
        co = bias.shape[0]
        arr = np.stack([(bias * (2.0 ** t)).reshape(co // 128, 128).T
                        for t in range(T)], axis=1)
        return np.ascontiguousarray(arr.astype(np.float32))

    feed = {}
    wstack = {}
    biases = np.zeros((128, T, 32), np.float32)
    bslot = {"qw": 0, "kw": 4, "pw": 8, "w2": 12, "w1": 16}
    for nm, wkey, bkey, bnkey in [("qw", "qw", "qb", "qbn"), ("kw", "kw", "kb", "kbn"),
                                  ("vw", "vw", "vb", "vbn"), ("pw", "pw", "pb", "pbn"),
                                  ("w1", "w1", "b1", "bn1"), ("w2", "w2", "b2", "bn2")]:
        W, bias = fold(inputs[wkey], inputs[bkey], inputs[bnkey])
        w_dt = np.float16 if nm in ("pw", "w2") else np.float32
        WT = W.T  # [ci, co]
        ci, co = WT.shape
        wl = np.ascontiguousarray(
            WT.reshape(ci // 128, 128, co).transpose(1, 0, 2).astype(w_dt))
        if nm in ("qw", "kw", "vw"):
            wstack[nm] = wl
        else:
            feed[nm] = wl
        if nm == "vw":
            vrow = np.zeros((1, C + 128), np.float32)
            vrow[0, :C] = bias.astype(np.float32)
            vrow[0, C:] = 1.0
            feed["vrow"] = vrow
        else:
            s = bslot[nm]
            biases[:, :, s:s + co // 128] = bias_layout(bias)
    feed["biases"] = biases
    feed["wqkv"] = np.ascontiguousarray(
        np.concatenate([wstack["qw"], wstack["kw"], wstack["vw"]], axis=1))

    gamma = 1.0 - 2.0 ** (-5.0 - np.arange(H, dtype=np.float64))
    idx = np.arange(N, dtype=np.float64)
    dist = np.abs(idx[:, None] - idx[None, :])
    scale = (C // H) ** -0.5
    dm = np.empty((H, 128, NT, N), np.float16)
    for h in range(H):
        dm[h] = ((gamma[h] ** dist) * scale).reshape(NT, 128, N).transpose(
            1, 0, 2).astype(np.float16)
    feed["dmat"] = dm
    return feed


def kernel(**inputs):
    if "nc" not in _CACHE:
        _CACHE["nc"] = _build()
    nc = _CACHE["nc"]
    feed = _host_prep(inputs)
    x = inputs["x"]
    in_maps = []
    for b in range(B):
        m = dict(feed)
        xt = x[:, b].transpose(0, 2, 1).reshape(T, CT, 128, N)
        m["xb"] = np.ascontiguousarray(xt)
        in_maps.append(m)
    res = None
    last_err = None
    for _attempt in range(3):
        try:
            res = run_bass_kernel_spmd(nc, in_maps, list(range(B)))
            break
        except Exception as e:  # transient NRT device wedges recover on retry
            last_err = e
    if res is None:
        raise last_err
    out = np.empty((T, B, N, C), np.float32)
    for b in range(B):
        oT = res.results[b]["out"].reshape(T, C, N).astype(np.float32)
        out[:, b] = oT.transpose(0, 2, 1)
    return out



# revision 14
# speedup vs baseline: 1.1930x; 1.1930x over previous
"""Spiking transformer block (SpikingRetention + spiking MLP) on 8 Trainium2
cores. Data-parallel over B=8 (one batch element per NeuronCore).

Key design (v2):
- Binary spikes are exact in fp8e4, enabling DoubleRow (double-pumped) PE
  matmuls at 0.5 cycles/row:
  * scores q.T k: stride-0 dim-2 APs compute 2*(k.T q) exactly; the decay
    matrix folds the 0.5.
  * retention out: real 2-chunk DoubleRow over m-tiles; s = scores*D in fp8.
  * proj / fc2: weights packed as [Q(W*s), e4m3-residual] chunk pairs with a
    stride-0 spike ifmap -> full-precision-ish weights at DoubleRow rate.
    Per-channel scales s_c keep the fp8 quantization near 2^-4 relative.
- LIF carry-adds ride the PE as scaled-identity accumulation matmuls into the
  next wave's psum group, freeing the vector engines.
- LIF per step: Act extract (A = 2^(t-1) psum + b~), DVE spike (is_ge), Pool
  carry (scalar_tensor_tensor). v/retention skip the extract: spike+carry read
  psum directly (no bias; thresholds constant in psum units).
- x stays resident in SBUF across both stages (loaded once).

Membrane algebra: A_t = 2^t u_t = A^r_{t-1} + 2^(t-1)(Wx_t + b). Carry
C = A (A < th 2^t); the consuming wave's psum gets 2^-(t-1) I @ C. For
direct (extract-free) chains the carry is stored in psum units and the
identity is 0.5 I for every t. proj/fc2 run entirely in per-channel-scaled
units (psum, bias, threshold, carry all scaled by s_c), so no rescale is
ever needed.
"""

from contextlib import ExitStack

import numpy as np
import ml_dtypes

import concourse.bacc as bacc
import concourse.tile as tile
from concourse import mybir
from concourse.bass_utils import run_bass_kernel_spmd

f32 = mybir.dt.float32
f32r = mybir.dt.float32r
fp16 = mybir.dt.float16
fp8 = mybir.dt.float8e4
Alu = mybir.AluOpType
Act = mybir.ActivationFunctionType
DR = mybir.MatmulPerfMode.DoubleRow

E4 = ml_dtypes.float8_e4m3

T, B, N, C = 4, 8, 512, 512
HID = 2048
H = 8
EPS = 1e-5
NT = N // 128
CT = C // 128
HT = HID // 128

_CACHE = {}


def _dr2(ap):
    """[p, f] -> [p, 2(stride0), f] for stride-0 DoubleRow operands."""
    p, fr = ap.shape
    return ap.unsqueeze(1).broadcast_to([p, 2, fr])


def _build():
    nc = bacc.Bacc("TRN2", target_bir_lowering=False, debug=False)

    xb = nc.declare_dram_parameter("xb", [T, CT, 128, N], f32r, isOutput=False)
    wqkv_e = nc.declare_dram_parameter("wqkv", [128, 3 * CT, C], f32r,
                                       isOutput=False)
    pw_e = nc.declare_dram_parameter("pw", [128, CT, 2, C], fp8, isOutput=False)
    w1_e = nc.declare_dram_parameter("w1", [128, CT, HID], f32r, isOutput=False)
    w2_e = nc.declare_dram_parameter("w2", [128, HT, 2, C], fp8, isOutput=False)
    bias_e = nc.declare_dram_parameter("biases", [128, T, 32], f32,
                                       isOutput=False)
    thr_e = nc.declare_dram_parameter("thrs", [128, T, 8], f32, isOutput=False)
    vb_e = nc.declare_dram_parameter("vrow", [1, C + 128], f32r, isOutput=False)
    dmat_e = nc.declare_dram_parameter("dmat", [H, 128, NT, N], fp16,
                                       isOutput=False)
    idm_e = nc.declare_dram_parameter("idm", [128, 3, 128], f32r,
                                      isOutput=False)
    out_e = nc.declare_dram_parameter("out", [T, CT, 128, N], fp16,
                                      isOutput=True)

    DVE = nc.vector
    POOL = nc.gpsimd
    ACT = nc.scalar

    with tile.TileContext(nc) as tc, ExitStack() as ctx:
        pers = ctx.enter_context(tc.tile_pool(name="pers", bufs=1))
        work = ctx.enter_context(tc.tile_pool(name="work", bufs=1))
        xa_pool = ctx.enter_context(tc.tile_pool(name="xa_pool", bufs=1))
        spk_o_pool = ctx.enter_context(tc.tile_pool(name="spk_o_pool", bufs=1))
        wmlp_pool = ctx.enter_context(tc.tile_pool(name="wmlp_pool", bufs=1))
        w1t = wmlp_pool.tile([128, CT, HID], f32r, name="w_w1")
        w2t = wmlp_pool.tile([128, HT, 2, C], fp8, name="w_w2")
        pwt = wmlp_pool.tile([128, CT, 2, C], fp8, name="w_pw")

        ball = pers.tile([128, T, 32], f32, name="ball")
        thrt = pers.tile([128, T, 8], f32, name="thrt")
        vrow = pers.tile([1, C + 128], f32r, name="vrow")
        idmt = pers.tile([128, 3, 128], f32r, name="idmt")
        ACT.dma_start(ball[:], bias_e[:, :, :])
        ACT.dma_start(thrt[:], thr_e[:, :, :])
        ACT.dma_start(vrow[:], vb_e[:, :])
        ACT.dma_start(idmt[:], idm_e[:, :, :])
        bias_sb = {"qb": ball[:, :, 0:4], "kb": ball[:, :, 4:8],
                   "pb": ball[:, :, 8:12], "b2": ball[:, :, 12:16],
                   "b1": ball[:, :, 16:32]}
        thr_sb = {"pb": thrt[:, :, 0:4], "b2": thrt[:, :, 4:8]}
        nthr = {}
        for tv in (1.0, 2.0):
            tt_ = pers.tile([128, 1], f32, name=f"nthr{int(tv)}")
            nc.vector.memset(tt_[:], -tv)
            nthr[tv] = tt_
        vbrow = vrow[:, 0:C]
        ones128 = vrow[:, C:C + 128]
        ids = {tt: idmt[:, tt - 2, :] for tt in (2, 3, 4)}


        os_ = {}
        decay_rr = [0]
        DEC_ENGS = (DVE, DVE, DVE, DVE, DVE, DVE, DVE, POOL)

        # ---------------- LIF helpers ----------------
        # spec: (src_ap, thr(imm or AP), spike_tile or None, sdt, cpool,
        #        ctag or None, carry_dst dict+key)
        def act_spike(st_ap, src, tv):
            # spike via Act (Relu with negated threshold, then Sign) to
            # offload DVE; exact for non-tie values like the baseline
            rl = work.tile([128, 512], f32, name="lifrl", tag="lifrl", bufs=2)
            ACT.activation(rl[:], src, Act.Relu, bias=nthr[tv][:, 0:1])
            ACT.activation(st_ap, rl[:], Act.Sign)

        def emit_spikes(specs):
            for (src, thr, st, cp, ctag, cdst, ckey) in specs:
                if st is not None:
                    DVE.tensor_scalar(st[:], src, thr, None, Alu.is_ge)

        def emit_carries(specs):
            for (src, thr, st, cp, ctag, cdst, ckey) in specs:
                if ctag is not None:
                    cn = cp.tile([128, 512], f32r, name="lifC", tag=ctag,
                                 bufs=1)
                    POOL.scalar_tensor_tensor(cn[:], src, thr, src,
                                              Alu.is_lt, Alu.mult)
                    cdst[ckey] = cn

        # =========== stage A: qkv + retention, t-outer wavefront ===========
        with tc.tile_pool(name="wqkv_pool", bufs=1) as wqkv_pool, \
             tc.tile_pool(name="spk_pool", bufs=1) as spk_pool, \
             tc.tile_pool(name="carry_pool", bufs=1) as carry_pool, \
             tc.tile_pool(name="dm_pool", bufs=1) as dm_pool, \
             tc.tile_pool(name="spool", bufs=1) as spool, \
             tc.tile_pool(name="psA", bufs=1, space="PSUM") as psA:
            wqkv_t = wqkv_pool.tile([128, 3 * CT, C], f32r, name="w_qkv")
            # startup: interleave x wave-1 chunks with qw chunks so the first
            # matmul can start after ~0.5MB of DMA; all on the Pool SWDGE
            # queue (served in emission order by the DMA device).
            xwt = xa_pool.tile([128, CT, N], f32r, name="xT", tag="xT", bufs=2)
            for kt in range(CT):
                POOL.dma_start(xwt[:, kt, :], xb[0, kt])
                POOL.dma_start(wqkv_t[:, kt, :], wqkv_e[:, kt, :])
            for kt in range(CT, 3 * CT):
                POOL.dma_start(wqkv_t[:, kt, :], wqkv_e[:, kt, :])
            wq = {nm: wqkv_t[:, i * CT:(i + 1) * CT, :]
                  for i, nm in enumerate(("qw", "kw", "vw"))}
            dmt = dm_pool.tile([128, H, NT, N], fp16, name="dmt")
            dms = [dmt[:, h] for h in range(H)]

            cq = {}     # carries for q/k/v chains, keyed (nm, ot)
            c_ret = {}  # retention carries per hp

            def ret_scores(hp, qs_p, ks_p, sdst):
                # per head pair: 8 stride-0 DR matmuls + 4 decay multiplies
                h0, h1 = 2 * hp, 2 * hp + 1
                for half in range(2):
                    ps0 = psA.tile([128, 2, N], f32, name="sc0", tag="sc0",
                                   bufs=1)
                    ps1 = psA.tile([128, 2, N], f32, name="sc1", tag="sc1",
                                   bufs=1)
                    for j in range(2):
                        mt = 2 * half + j
                        nc.tensor.matmul(
                            ps0[:, j, :],
                            _dr2(ks_p[hp][0:64, mt * 128:(mt + 1) * 128]),
                            _dr2(qs_p[hp][0:64, :]),
                            start=True, stop=True, perf_mode=DR)
                        nc.tensor.matmul(
                            ps1[:, j, :],
                            _dr2(ks_p[hp][64:128, mt * 128:(mt + 1) * 128]),
                            _dr2(qs_p[hp][64:128, :]),
                            start=True, stop=True, perf_mode=DR)
                    s0 = spool.tile([128, 2, N], fp8, name="sd0", tag="sd0",
                                    bufs=3)
                    e0 = DEC_ENGS[decay_rr[0] % len(DEC_ENGS)]
                    decay_rr[0] += 1
                    e0.tensor_tensor(s0[:], ps0[:],
                                     dms[h0][:, 2 * half:2 * half + 2, :],
                                     Alu.mult)
                    s1 = spool.tile([128, 2, N], fp8, name="sd1", tag="sd1",
                                    bufs=3)
                    e1 = DEC_ENGS[decay_rr[0] % len(DEC_ENGS)]
                    decay_rr[0] += 1
                    e1.tensor_tensor(s1[:], ps1[:],
                                     dms[h1][:, 2 * half:2 * half + 2, :],
                                     Alu.mult)
                    sdst[hp, half] = (s0, s1)

            def ret_out(hp, sdst, vt_p, t_r):
                h0, h1 = 2 * hp, 2 * hp + 1
                pso = psA.tile([128, N], f32, name="pso", tag="pso", bufs=1)
                has_c = (hp in c_ret)
                for half in range(2):
                    s0, s1 = sdst.pop((hp, half))
                    last = (half == 1) and not has_c
                    nc.tensor.matmul(
                        pso[0:64, :],
                        vt_p[:, 2 * half:2 * half + 2,
                             h0 * 64:(h0 + 1) * 64],
                        s0[:], start=(half == 0), stop=last, perf_mode=DR)
                    nc.tensor.matmul(
                        pso[64:128, :],
                        vt_p[:, 2 * half:2 * half + 2,
                             h1 * 64:(h1 + 1) * 64],
                        s1[:], start=(half == 0), stop=last, perf_mode=DR)
                if has_c:
                    nc.tensor.matmul(pso[:], ids[2], c_ret[hp][:],
                                     start=False, stop=True)
                st = spk_o_pool.tile([128, N], fp8, name="spk_os",
                                     tag="spk_os", bufs=16)
                os_[t_r - 1, hp] = st
                act_spike(st[:], pso[:], 1.0)
                spec = (pso[:], 1.0, None, spool,
                        f"c_o{hp}" if t_r < T else None, c_ret, hp)
                emit_carries([spec])

            prev = None
            xw_next = None
            for t in range(1, T + 1):
                if t > 1:
                    xwt = xw_next
                xw = {ct: xwt[:, ct, :] for ct in range(CT)}
                qs_c = {}
                ks_c = {}
                vt = spk_pool.tile([128, NT, C], fp8, name="vn", tag="vn",
                                   bufs=2)
                specs = []

                def emit_qk(nm, bnm, dst, ot, t=t):
                    ps = psA.tile([128, N], f32, name="psq", tag="psq", bufs=3)
                    cin = cq.get((nm, ot))
                    for kt in range(CT):
                        nc.tensor.matmul(
                            ps[:], wq[nm][:, kt, ot * 128:(ot + 1) * 128],
                            xw[kt], start=(kt == 0),
                            stop=(kt == CT - 1) and cin is None)
                    if cin is not None:
                        nc.tensor.matmul(ps[:], ids[t], cin[:],
                                         start=False, stop=True)
                    A = work.tile([128, 512], f32, name="lifA", tag="lifA",
                                  bufs=6)
                    ACT.activation(A[:], ps[:], Act.Identity,
                                   bias=bias_sb[bnm][:, t - 1, ot:ot + 1],
                                   scale=float(2.0 ** (t - 1)))
                    st = spk_pool.tile([128, N], fp8, name=f"spk_{nm}",
                                       tag=f"spk_{nm}", bufs=8)
                    dst[ot] = st
                    specs.append((A[:], float(2.0 ** t), st, carry_pool,
                                  f"c_{nm}{ot}" if t < T else None, cq,
                                  (nm, ot)))

                def emit_v(nt, t=t):
                    ps = psA.tile([128, C], f32, name="psv", tag="psq", bufs=3)
                    cin = cq.get(("vw", nt))
                    for kt in range(CT):
                        nc.tensor.matmul(ps[:],
                                         xw[kt][:, nt * 128:(nt + 1) * 128],
                                         wq["vw"][:, kt, :],
                                         start=(kt == 0), stop=False)
                    nc.tensor.matmul(ps[:], ones128, vbrow,
                                     start=False, stop=cin is None)
                    if cin is not None:
                        nc.tensor.matmul(ps[:], ids[2], cin[:],
                                         start=False, stop=True)
                    # direct LIF on psum: A = 2^(t-1) psum -> spike at 2.0
                    act_spike(vt[:, nt, :], ps[:], 2.0)
                    spec = (ps[:], 2.0, None, carry_pool,
                            f"c_vw{nt}" if t < T else None, cq, ("vw", nt))
                    emit_carries([spec])

                groups = [lambda ot=ot: emit_qk("qw", "qb", qs_c, ot)
                          for ot in range(CT)]
                groups += [lambda ot=ot: emit_qk("kw", "kb", ks_c, ot)
                           for ot in range(CT)]
                groups += [lambda nt=nt: emit_v(nt) for nt in range(NT)]

                if prev is not None:
                    qs_p, ks_p, vt_p = prev
                    sd = {}
                    order = [0, 1, 2, ("s", 0), 3, 4, ("o", 0), ("s", 1),
                             5, 6, ("o", 1), ("s", 2), 7, 8, ("o", 2),
                             ("s", 3), 9, 10, ("o", 3), 11]
                    for item in order:
                        if isinstance(item, int):
                            groups[item]()
                        elif item[0] == "s":
                            ret_scores(item[1], qs_p, ks_p, sd)
                        else:
                            ret_out(item[1], sd, vt_p, t - 1)
                else:
                    for g in groups:
                        g()
                emit_spikes(specs)
                emit_carries(specs)
                if t < T:  # prefetch next wave's x (SP hwdge queue: free)
                    xw_next = xa_pool.tile([128, CT, N], f32r, name="xT",
                                           tag="xT", bufs=2)
                    for kt in range(CT):
                        nc.sync.dma_start(xw_next[:, kt, :], xb[t, kt])
                if t == 1:  # decay matrices after wave-2 x
                    for hp in range(4):
                        POOL.dma_start(
                            dmt[:, 2 * hp:2 * hp + 2],
                            dmat_e.rearrange("h p nt n -> p h nt n")
                            [:, 2 * hp:2 * hp + 2])
                # stage-B weights trickle in on the sync (SP hwdge) queue,
                # spread across wave boundaries so they never crowd out the
                # x prefetches on the shared DMA device
                if t == 1:
                    nc.sync.dma_start(pwt[:], pw_e[:, :, :, :])
                    for kt in range(2):
                        nc.sync.dma_start(w1t[:, kt], w1_e[:, kt])
                elif t == 2:
                    for kt in range(2, CT):
                        nc.sync.dma_start(w1t[:, kt], w1_e[:, kt])
                    for ktp in range(2):
                        nc.sync.dma_start(w2t[:, 4 * ktp:4 * ktp + 4],
                                          w2_e[:, 4 * ktp:4 * ktp + 4])
                elif t == 3:
                    for ktp in range(2, 4):
                        nc.sync.dma_start(w2t[:, 4 * ktp:4 * ktp + 4],
                                          w2_e[:, 4 * ktp:4 * ktp + 4])
                prev = (qs_c, ks_c, vt)
            # final retention wave (t = T)
            qs_p, ks_p, vt_p = prev
            sd = {}
            for hp in range(H // 2):
                ret_scores(hp, qs_p, ks_p, sd)
                ret_out(hp, sd, vt_p, T)

        # =========== stage B: proj + MLP + output ===========
        with tc.tile_pool(name="mwork", bufs=1) as mwork, \
             tc.tile_pool(name="xtin_pool", bufs=1) as xtin_pool, \
             tc.tile_pool(name="psM", bufs=1, space="PSUM") as psM:
            cp = {}
            c1 = {}
            c2 = {}
            x2_all = {}
            ht_all = {}

            def fc2_wave(t):
                htile = ht_all.pop(t)
                x2 = x2_all.pop(t)
                last = (t == T)
                specs = []
                sts = []
                for ot in range(CT):
                    ps = psM.tile([128, N], f32, name="psf2", tag="psf2",
                                  bufs=2)
                    cin = c2.get(ot)
                    for kt in range(HT):
                        nc.tensor.matmul(
                            ps[:], w2t[:, kt, :, ot * 128:(ot + 1) * 128],
                            _dr2(htile[kt][:]), start=(kt == 0),
                            stop=(kt == HT - 1) and cin is None, perf_mode=DR)
                    if cin is not None:
                        nc.tensor.matmul(ps[:], ids[t], cin[:],
                                         start=False, stop=True)
                    A = work.tile([128, 512], f32, name="lifA", tag="lifA",
                                  bufs=6)
                    ACT.activation(A[:], ps[:], Act.Identity,
                                   bias=bias_sb["b2"][:, t - 1, ot:ot + 1],
                                   scale=float(2.0 ** (t - 1)))
                    st = mwork.tile([128, N], fp16, name="spk_m", tag="spk_m",
                                    bufs=2)
                    sts.append(st)
                    specs.append((A[:], thr_sb["b2"][:, t - 1, ot:ot + 1], st,
                                  mwork, f"c2_{ot}" if t < T else None, c2, ot))
                emit_spikes(specs)
                emit_carries(specs)
                outb = mwork.tile([128, CT, N], fp16, name="outb", tag="outb",
                                  bufs=1)
                for ot in range(CT):
                    DVE.tensor_tensor(outb[:, ot, :], x2[ot], sts[ot][:],
                                      Alu.add)
                    if last:
                        nc.sync.dma_start(out_e[t - 1, ot], outb[:, ot, :])
                if not last:
                    ACT.dma_start(
                        out_e[t - 1].rearrange("ct p n -> p ct n"), outb[:])

            for t in range(1, T + 1):
                xin = xtin_pool.tile([128, CT, N], f32r, name="xtin",
                                     tag="xtin", bufs=2)
                for kt in range(CT):
                    nc.sync.dma_start(xin[:, kt, :], xb[t - 1, kt])
                # proj: DR residual weights, fp8 retention-spike ifmap
                x2 = {}
                specs = []
                stps = []
                for ot in range(CT):
                    ps = psM.tile([128, N], f32, name="psp", tag="psp", bufs=2)
                    cin = cp.get(ot)
                    for kt in range(CT):
                        nc.tensor.matmul(
                            ps[:], pwt[:, kt, :, ot * 128:(ot + 1) * 128],
                            _dr2(os_[t - 1, kt][:]), start=(kt == 0),
                            stop=(kt == CT - 1) and cin is None, perf_mode=DR)
                    if cin is not None:
                        nc.tensor.matmul(ps[:], ids[t], cin[:],
                                         start=False, stop=True)
                    A = work.tile([128, 512], f32, name="lifA", tag="lifA",
                                  bufs=6)
                    ACT.activation(A[:], ps[:], Act.Identity,
                                   bias=bias_sb["pb"][:, t - 1, ot:ot + 1],
                                   scale=float(2.0 ** (t - 1)))
                    stp = mwork.tile([128, N], fp16, name="spk_p",
                                     tag="spk_p", bufs=2)
                    stps.append(stp)
                    specs.append((A[:], thr_sb["pb"][:, t - 1, ot:ot + 1],
                                  stp, mwork, f"cp_{ot}" if t < T else None,
                                  cp, ot))
                emit_spikes(specs)
                emit_carries(specs)
                x2b = mwork.tile([128, CT, N], f32r, name="x2t", tag="x2t",
                                 bufs=2)
                for ot in range(CT):
                    DVE.tensor_tensor(x2b[:, ot, :], xin[:, ot, :],
                                      stps[ot][:], Alu.add)
                    x2[ot] = x2b[:, ot, :]
                x2_all[t] = x2
                if t > 1:
                    fc2_wave(t - 1)
                htile = {}
                specs = []
                for ot in range(HT):
                    ps = psM.tile([128, N], f32, name="psf1", tag="psf1",
                                  bufs=4)
                    cin = c1.get(ot)
                    for kt in range(CT):
                        nc.tensor.matmul(
                            ps[:], w1t[:, kt, ot * 128:(ot + 1) * 128],
                            x2[kt], start=(kt == 0),
                            stop=(kt == CT - 1) and cin is None)
                    if cin is not None:
                        nc.tensor.matmul(ps[:], ids[t], cin[:],
                                         start=False, stop=True)
                    A = work.tile([128, 512], f32, name="lifA", tag="lifA",
                                  bufs=6)
                    ACT.activation(A[:], ps[:], Act.Identity,
                                   bias=bias_sb["b1"][:, t - 1, ot:ot + 1],
                                   scale=float(2.0 ** (t - 1)))
                    st = mwork.tile([128, N], fp8, name="spk_h", tag="spk_h",
                                    bufs=HT)
                    htile[ot] = st
                    specs.append((A[:], float(2.0 ** t), st, mwork,
                                  f"c1_{ot}" if t < T else None, c1, ot))
                    if ot % 4 == 3:  # flush quarters to bound A-tile usage
                        emit_spikes(specs)
                        emit_carries(specs)
                        specs = []
                ht_all[t] = htile
            fc2_wave(T)

    nc.finalize()
    return nc


def _host_prep(inputs):
    def fold(w, b, bn):
        g, bb, m, v = [bn[i].astype(np.float64) for i in range(4)]
        A = g / np.sqrt(v + EPS)
        W = w.astype(np.float64) * A[:, None]
        bias = (b.astype(np.float64) - m) * A + bb
        return W, bias

    def col_layout(WT, dt):
        # [ci, co] -> [128, ci//128, co]
        ci, co = WT.shape
        return np.ascontiguousarray(
            WT.reshape(ci // 128, 128, co).transpose(1, 0, 2).astype(dt))

    def bias_layout(vals):
        # per-t scaled rows: [co] -> [128, T, co//128]
        co = vals.shape[0]
        arr = np.stack([(vals * (2.0 ** t)).reshape(co // 128, 128).T
                        for t in range(T)], axis=1)
        return np.ascontiguousarray(arr.astype(np.float32))

    feed = {}
    wstack = {}
    biases = np.zeros((128, T, 32), np.float32)
    thrs = np.zeros((128, T, 8), np.float32)
    bslot = {"qw": 0, "kw": 4, "pw": 8, "w2": 12, "w1": 16}
    tslot = {"pw": 0, "w2": 4}
    for nm, bkey, bnkey in [("qw", "qb", "qbn"), ("kw", "kb", "kbn"),
                            ("vw", "vb", "vbn"), ("pw", "pb", "pbn"),
                            ("w1", "b1", "bn1"), ("w2", "b2", "bn2")]:
        W, bias = fold(inputs[nm], inputs[bkey], inputs[bnkey])
        if nm == "vw":
            feed["wqkv_v"] = col_layout(W.T, np.float32)
            vrow = np.zeros((1, C + 128), np.float32)
            vrow[0, :C] = bias.astype(np.float32)
            vrow[0, C:] = 1.0
            feed["vrow"] = vrow
            continue
        if nm in ("pw", "w2"):
            # per-channel scale, e4m3 + residual packing
            s = 1.0 / np.sqrt(np.mean(W ** 2, axis=1))
            Ws = (W * s[:, None]).astype(np.float32)
            WT = Ws.T  # [ci, co]
            ci, co = WT.shape
            Q = WT.astype(E4)
            R = (WT - Q.astype(np.float32)).astype(E4)
            pair = np.stack([Q, R], axis=1)  # [ci, 2, co]
            feed[nm] = np.ascontiguousarray(
                pair.reshape(ci // 128, 128, 2, co)
                .transpose(1, 0, 2, 3).astype(E4))
            # extract bias = s_c*b*2^(t-1); spike thr = s_c*2^t = (2 s_c)*2^(t-1)
            sb = (bias * s).astype(np.float64)
            biases[:, :, bslot[nm]:bslot[nm] + co // 128] = bias_layout(sb)
            thrs[:, :, tslot[nm]:tslot[nm] + co // 128] = \
                bias_layout(2.0 * s.astype(np.float64))
            continue
        wl = col_layout(W.T, np.float32)
        if nm in ("qw", "kw"):
            wstack[nm] = wl
        else:
            feed[nm] = wl
        co = bias.shape[0]
        biases[:, :, bslot[nm]:bslot[nm] + co // 128] = bias_layout(bias)
    feed["biases"] = biases
    feed["thrs"] = thrs
    feed["wqkv"] = np.ascontiguousarray(
        np.concatenate([wstack["qw"], wstack["kw"], feed.pop("wqkv_v")],
                       axis=1))

    gamma = 1.0 - 2.0 ** (-5.0 - np.arange(H, dtype=np.float64))
    idx = np.arange(N, dtype=np.float64)
    dist = np.abs(idx[:, None] - idx[None, :])
    scale = (C // H) ** -0.5
    dm = np.empty((H, 128, NT, N), np.float16)
    for h in range(H):
        dm[h] = ((gamma[h] ** dist) * scale * 0.5).reshape(
            NT, 128, N).transpose(1, 0, 2).astype(np.float16)
    feed["dmat"] = dm

    idm = np.zeros((128, 3, 128), np.float32)
    for i, sc in enumerate((0.5, 0.25, 0.125)):
        idm[:, i, :] = sc * np.eye(128, dtype=np.float32)
    feed["idm"] = idm
    return feed


def kernel(**inputs):
    if "nc" not in _CACHE:
        _CACHE["nc"] = _build()
    nc = _CACHE["nc"]
    feed = _host_prep(inputs)
    x = inputs["x"]
    in_maps = []
    for b in range(B):
        m = dict(feed)
        xt = x[:, b].transpose(0, 2, 1).reshape(T, CT, 128, N)
        m["xb"] = np.ascontiguousarray(xt)
        in_maps.append(m)
    res = None
    last_err = None
    for _attempt in range(3):
        try:
            res = run_bass_kernel_spmd(nc, in_maps, list(range(B)))
            break
        except Exception as e:  # transient NRT device wedges recover on retry
            last_err = e
    if res is None:
        raise last_err
    out = np.empty((T, B, N, C), np.float32)
    for b in range(B):
        oT = res.results[b]["out"].reshape(T, C, N).astype(np.float32)
        out[:, b] = oT.transpose(0, 2, 1)
    return out


# revision 15
# speedup vs baseline: 1.1937x; 1.0005x over previous
"""Spiking transformer block (SpikingRetention + spiking MLP) on 8 Trainium2
cores. Data-parallel over B=8 (one batch element per NeuronCore).

Key design (v2):
- Binary spikes are exact in fp8e4, enabling DoubleRow (double-pumped) PE
  matmuls at 0.5 cycles/row:
  * scores q.T k: stride-0 dim-2 APs compute 2*(k.T q) exactly; the decay
    matrix folds the 0.5.
  * retention out: real 2-chunk DoubleRow over m-tiles; s = scores*D in fp8.
  * proj / fc2: weights packed as [Q(W*s), e4m3-residual] chunk pairs with a
    stride-0 spike ifmap -> full-precision-ish weights at DoubleRow rate.
    Per-channel scales s_c keep the fp8 quantization near 2^-4 relative.
- LIF carry-adds ride the PE as scaled-identity accumulation matmuls into the
  next wave's psum group, freeing the vector engines.
- LIF per step: Act extract (A = 2^(t-1) psum + b~), DVE spike (is_ge), Pool
  carry (scalar_tensor_tensor). v/retention skip the extract: spike+carry read
  psum directly (no bias; thresholds constant in psum units).
- x stays resident in SBUF across both stages (loaded once).

Membrane algebra: A_t = 2^t u_t = A^r_{t-1} + 2^(t-1)(Wx_t + b). Carry
C = A (A < th 2^t); the consuming wave's psum gets 2^-(t-1) I @ C. For
direct (extract-free) chains the carry is stored in psum units and the
identity is 0.5 I for every t. proj/fc2 run entirely in per-channel-scaled
units (psum, bias, threshold, carry all scaled by s_c), so no rescale is
ever needed.
"""

from contextlib import ExitStack

import numpy as np
import ml_dtypes

import concourse.bacc as bacc
import concourse.tile as tile
from concourse import mybir
from concourse.bass_utils import run_bass_kernel_spmd

f32 = mybir.dt.float32
f32r = mybir.dt.float32r
fp16 = mybir.dt.float16
fp8 = mybir.dt.float8e4
Alu = mybir.AluOpType
Act = mybir.ActivationFunctionType
DR = mybir.MatmulPerfMode.DoubleRow

E4 = ml_dtypes.float8_e4m3

T, B, N, C = 4, 8, 512, 512
HID = 2048
H = 8
EPS = 1e-5
NT = N // 128
CT = C // 128
HT = HID // 128

_CACHE = {}


def _dr2(ap):
    """[p, f] -> [p, 2(stride0), f] for stride-0 DoubleRow operands."""
    p, fr = ap.shape
    return ap.unsqueeze(1).broadcast_to([p, 2, fr])


def _build():
    nc = bacc.Bacc("TRN2", target_bir_lowering=False, debug=False)

    xb = nc.declare_dram_parameter("xb", [T, CT, 128, N], f32r, isOutput=False)
    wqkv_e = nc.declare_dram_parameter("wqkv", [128, 3 * CT, C], f32r,
                                       isOutput=False)
    pw_e = nc.declare_dram_parameter("pw", [128, CT, 2, C], fp8, isOutput=False)
    w1_e = nc.declare_dram_parameter("w1", [128, CT, HID], f32r, isOutput=False)
    w2_e = nc.declare_dram_parameter("w2", [128, HT, 2, C], fp8, isOutput=False)
    bias_e = nc.declare_dram_parameter("biases", [128, T, 32], f32,
                                       isOutput=False)
    thr_e = nc.declare_dram_parameter("thrs", [128, T, 8], f32, isOutput=False)
    vb_e = nc.declare_dram_parameter("vrow", [1, C + 128], f32r, isOutput=False)
    dmat_e = nc.declare_dram_parameter("dmat", [H, 128, NT, N], fp16,
                                       isOutput=False)
    idm_e = nc.declare_dram_parameter("idm", [128, 3, 128], f32r,
                                      isOutput=False)
    out_e = nc.declare_dram_parameter("out", [T, CT, 128, N], fp16,
                                      isOutput=True)

    DVE = nc.vector
    POOL = nc.gpsimd
    ACT = nc.scalar

    with tile.TileContext(nc) as tc, ExitStack() as ctx:
        pers = ctx.enter_context(tc.tile_pool(name="pers", bufs=1))
        work = ctx.enter_context(tc.tile_pool(name="work", bufs=1))
        xa_pool = ctx.enter_context(tc.tile_pool(name="xa_pool", bufs=1))
        spk_o_pool = ctx.enter_context(tc.tile_pool(name="spk_o_pool", bufs=1))
        wmlp_pool = ctx.enter_context(tc.tile_pool(name="wmlp_pool", bufs=1))
        w1t = wmlp_pool.tile([128, CT, HID], f32r, name="w_w1")
        w2t = wmlp_pool.tile([128, HT, 2, C], fp8, name="w_w2")
        pwt = wmlp_pool.tile([128, CT, 2, C], fp8, name="w_pw")

        ball = pers.tile([128, T, 32], f32, name="ball")
        thrt = pers.tile([128, T, 8], f32, name="thrt")
        vrow = pers.tile([1, C + 128], f32r, name="vrow")
        idmt = pers.tile([128, 3, 128], f32r, name="idmt")
        ACT.dma_start(ball[:], bias_e[:, :, :])
        ACT.dma_start(thrt[:], thr_e[:, :, :])
        ACT.dma_start(vrow[:], vb_e[:, :])
        ACT.dma_start(idmt[:], idm_e[:, :, :])
        bias_sb = {"qb": ball[:, :, 0:4], "kb": ball[:, :, 4:8],
                   "pb": ball[:, :, 8:12], "b2": ball[:, :, 12:16],
                   "b1": ball[:, :, 16:32]}
        thr_sb = {"pb": thrt[:, :, 0:4], "b2": thrt[:, :, 4:8]}
        nthr = {}
        for tv in (1.0, 2.0, 4.0, 8.0, 16.0):
            tt_ = pers.tile([128, 1], f32, name=f"nthr{int(tv)}")
            nc.vector.memset(tt_[:], -tv)
            nthr[tv] = tt_
        vbrow = vrow[:, 0:C]
        ones128 = vrow[:, C:C + 128]
        ids = {tt: idmt[:, tt - 2, :] for tt in (2, 3, 4)}


        os_ = {}
        decay_rr = [0]
        DEC_ENGS = (DVE,)

        # ---------------- LIF helpers ----------------
        # spec: (src_ap, thr(imm or AP), spike_tile or None, sdt, cpool,
        #        ctag or None, carry_dst dict+key)
        def act_spike(st_ap, src, tv):
            # spike via Act (Relu with negated threshold, then Sign) to
            # offload DVE; exact for non-tie values like the baseline
            rl = work.tile([128, 512], f32, name="lifrl", tag="lifrl", bufs=2)
            ACT.activation(rl[:], src, Act.Relu, bias=nthr[tv][:, 0:1])
            ACT.activation(st_ap, rl[:], Act.Sign)

        def emit_spikes(specs):
            for (src, thr, st, cp, ctag, cdst, ckey) in specs:
                if st is not None:
                    DVE.tensor_scalar(st[:], src, thr, None, Alu.is_ge)

        def emit_carries(specs):
            for (src, thr, st, cp, ctag, cdst, ckey) in specs:
                if ctag is not None:
                    cn = cp.tile([128, 512], f32r, name="lifC", tag=ctag,
                                 bufs=1)
                    POOL.scalar_tensor_tensor(cn[:], src, thr, src,
                                              Alu.is_lt, Alu.mult)
                    cdst[ckey] = cn

        # =========== stage A: qkv + retention, t-outer wavefront ===========
        with tc.tile_pool(name="wqkv_pool", bufs=1) as wqkv_pool, \
             tc.tile_pool(name="spk_pool", bufs=1) as spk_pool, \
             tc.tile_pool(name="carry_pool", bufs=1) as carry_pool, \
             tc.tile_pool(name="dm_pool", bufs=1) as dm_pool, \
             tc.tile_pool(name="spool", bufs=1) as spool, \
             tc.tile_pool(name="psA", bufs=1, space="PSUM") as psA:
            wqkv_t = wqkv_pool.tile([128, 3 * CT, C], f32r, name="w_qkv")
            # startup: interleave x wave-1 chunks with qw chunks so the first
            # matmul can start after ~0.5MB of DMA; all on the Pool SWDGE
            # queue (served in emission order by the DMA device).
            xwt = xa_pool.tile([128, CT, N], f32r, name="xT", tag="xT", bufs=2)
            for kt in range(CT):
                POOL.dma_start(xwt[:, kt, :], xb[0, kt])
                POOL.dma_start(wqkv_t[:, kt, :], wqkv_e[:, kt, :])
            for kt in range(CT, 3 * CT):
                POOL.dma_start(wqkv_t[:, kt, :], wqkv_e[:, kt, :])
            wq = {nm: wqkv_t[:, i * CT:(i + 1) * CT, :]
                  for i, nm in enumerate(("qw", "kw", "vw"))}
            dmt = dm_pool.tile([128, H, NT, N], fp16, name="dmt")
            dms = [dmt[:, h] for h in range(H)]

            cq = {}     # carries for q/k/v chains, keyed (nm, ot)
            c_ret = {}  # retention carries per hp

            def ret_scores(hp, qs_p, ks_p, sdst):
                # per head pair: 8 stride-0 DR matmuls + 4 decay multiplies
                h0, h1 = 2 * hp, 2 * hp + 1
                for half in range(2):
                    ps0 = psA.tile([128, 2, N], f32, name="sc0", tag="sc0",
                                   bufs=1)
                    ps1 = psA.tile([128, 2, N], f32, name="sc1", tag="sc1",
                                   bufs=1)
                    for j in range(2):
                        mt = 2 * half + j
                        nc.tensor.matmul(
                            ps0[:, j, :],
                            _dr2(ks_p[hp][0:64, mt * 128:(mt + 1) * 128]),
                            _dr2(qs_p[hp][0:64, :]),
                            start=True, stop=True, perf_mode=DR)
                        nc.tensor.matmul(
                            ps1[:, j, :],
                            _dr2(ks_p[hp][64:128, mt * 128:(mt + 1) * 128]),
                            _dr2(qs_p[hp][64:128, :]),
                            start=True, stop=True, perf_mode=DR)
                    s0 = spool.tile([128, 2, N], fp8, name="sd0", tag="sd0",
                                    bufs=3)
                    e0 = DEC_ENGS[decay_rr[0] % len(DEC_ENGS)]
                    decay_rr[0] += 1
                    e0.tensor_tensor(s0[:], ps0[:],
                                     dms[h0][:, 2 * half:2 * half + 2, :],
                                     Alu.mult)
                    s1 = spool.tile([128, 2, N], fp8, name="sd1", tag="sd1",
                                    bufs=3)
                    e1 = DEC_ENGS[decay_rr[0] % len(DEC_ENGS)]
                    decay_rr[0] += 1
                    e1.tensor_tensor(s1[:], ps1[:],
                                     dms[h1][:, 2 * half:2 * half + 2, :],
                                     Alu.mult)
                    sdst[hp, half] = (s0, s1)

            def ret_out(hp, sdst, vt_p, t_r):
                h0, h1 = 2 * hp, 2 * hp + 1
                pso = psA.tile([128, N], f32, name="pso", tag="pso", bufs=1)
                has_c = (hp in c_ret)
                for half in range(2):
                    s0, s1 = sdst.pop((hp, half))
                    last = (half == 1) and not has_c
                    nc.tensor.matmul(
                        pso[0:64, :],
                        vt_p[:, 2 * half:2 * half + 2,
                             h0 * 64:(h0 + 1) * 64],
                        s0[:], start=(half == 0), stop=last, perf_mode=DR)
                    nc.tensor.matmul(
                        pso[64:128, :],
                        vt_p[:, 2 * half:2 * half + 2,
                             h1 * 64:(h1 + 1) * 64],
                        s1[:], start=(half == 0), stop=last, perf_mode=DR)
                if has_c:
                    nc.tensor.matmul(pso[:], ids[t_r], c_ret[hp][:],
                                     start=False, stop=True)
                st = spk_o_pool.tile([128, N], fp8, name="spk_os",
                                     tag="spk_os", bufs=16)
                os_[t_r - 1, hp] = st
                A = work.tile([128, 512], f32, name="lifA", tag="lifA",
                              bufs=6)
                ACT.activation(A[:], pso[:], Act.Copy, bias=0.0,
                               scale=float(2.0 ** (t_r - 1)))
                act_spike(st[:], A[:], float(2.0 ** (t_r - 1)))
                spec = (A[:], float(2.0 ** (t_r - 1)), None, spool,
                        f"c_o{hp}" if t_r < T else None, c_ret, hp)
                emit_carries([spec])

            prev = None
            xw_next = None
            for t in range(1, T + 1):
                if t > 1:
                    xwt = xw_next
                xw = {ct: xwt[:, ct, :] for ct in range(CT)}
                qs_c = {}
                ks_c = {}
                vt = spk_pool.tile([128, NT, C], fp8, name="vn", tag="vn",
                                   bufs=2)
                specs = []

                def emit_qk(nm, bnm, dst, ot, t=t):
                    ps = psA.tile([128, N], f32, name="psq", tag="psq", bufs=3)
                    cin = cq.get((nm, ot))
                    for kt in range(CT):
                        nc.tensor.matmul(
                            ps[:], wq[nm][:, kt, ot * 128:(ot + 1) * 128],
                            xw[kt], start=(kt == 0),
                            stop=(kt == CT - 1) and cin is None)
                    if cin is not None:
                        nc.tensor.matmul(ps[:], ids[t], cin[:],
                                         start=False, stop=True)
                    A = work.tile([128, 512], f32, name="lifA", tag="lifA",
                                  bufs=6)
                    ACT.activation(A[:], ps[:], Act.Identity,
                                   bias=bias_sb[bnm][:, t - 1, ot:ot + 1],
                                   scale=float(2.0 ** (t - 1)))
                    st = spk_pool.tile([128, N], fp8, name=f"spk_{nm}",
                                       tag=f"spk_{nm}", bufs=8)
                    dst[ot] = st
                    specs.append((A[:], float(2.0 ** t), st, carry_pool,
                                  f"c_{nm}{ot}" if t < T else None, cq,
                                  (nm, ot)))

                def emit_v(nt, t=t):
                    ps = psA.tile([128, C], f32, name="psv", tag="psq", bufs=3)
                    cin = cq.get(("vw", nt))
                    for kt in range(CT):
                        nc.tensor.matmul(ps[:],
                                         xw[kt][:, nt * 128:(nt + 1) * 128],
                                         wq["vw"][:, kt, :],
                                         start=(kt == 0), stop=False)
                    nc.tensor.matmul(ps[:], ones128, vbrow,
                                     start=False, stop=cin is None)
                    if cin is not None:
                        nc.tensor.matmul(ps[:], ids[t], cin[:],
                                         start=False, stop=True)
                    A = work.tile([128, 512], f32, name="lifA", tag="lifA",
                                  bufs=6)
                    ACT.activation(A[:], ps[:], Act.Copy, bias=0.0,
                                   scale=float(2.0 ** (t - 1)))
                    act_spike(vt[:, nt, :], A[:], float(2.0 ** t))
                    spec = (A[:], float(2.0 ** t), None, carry_pool,
                            f"c_vw{nt}" if t < T else None, cq, ("vw", nt))
                    emit_carries([spec])

                groups = [lambda ot=ot: emit_qk("qw", "qb", qs_c, ot)
                          for ot in range(CT)]
                groups += [lambda ot=ot: emit_qk("kw", "kb", ks_c, ot)
                           for ot in range(CT)]
                groups += [lambda nt=nt: emit_v(nt) for nt in range(NT)]

                if prev is not None:
                    qs_p, ks_p, vt_p = prev
                    sd = {}
                    order = [0, 1, 2, ("s", 0), 3, 4, ("o", 0), ("s", 1),
                             5, 6, ("o", 1), ("s", 2), 7, 8, ("o", 2),
                             ("s", 3), 9, 10, ("o", 3), 11]
                    for item in order:
                        if isinstance(item, int):
                            groups[item]()
                        elif item[0] == "s":
                            ret_scores(item[1], qs_p, ks_p, sd)
                        else:
                            ret_out(item[1], sd, vt_p, t - 1)
                else:
                    for g in groups:
                        g()
                emit_spikes(specs)
                emit_carries(specs)
                if t < T:  # prefetch next wave's x (SP hwdge queue: free)
                    xw_next = xa_pool.tile([128, CT, N], f32r, name="xT",
                                           tag="xT", bufs=2)
                    for kt in range(CT):
                        nc.sync.dma_start(xw_next[:, kt, :], xb[t, kt])
                if t == 1:  # decay matrices after wave-2 x
                    for hp in range(4):
                        POOL.dma_start(
                            dmt[:, 2 * hp:2 * hp + 2],
                            dmat_e.rearrange("h p nt n -> p h nt n")
                            [:, 2 * hp:2 * hp + 2])
                # stage-B weights trickle in on the sync (SP hwdge) queue,
                # spread across wave boundaries so they never crowd out the
                # x prefetches on the shared DMA device
                if t == 1:
                    nc.sync.dma_start(pwt[:], pw_e[:, :, :, :])
                    for kt in range(2):
                        nc.sync.dma_start(w1t[:, kt], w1_e[:, kt])
                elif t == 2:
                    for kt in range(2, CT):
                        nc.sync.dma_start(w1t[:, kt], w1_e[:, kt])
                    for ktp in range(2):
                        nc.sync.dma_start(w2t[:, 4 * ktp:4 * ktp + 4],
                                          w2_e[:, 4 * ktp:4 * ktp + 4])
                elif t == 3:
                    for ktp in range(2, 4):
                        nc.sync.dma_start(w2t[:, 4 * ktp:4 * ktp + 4],
                                          w2_e[:, 4 * ktp:4 * ktp + 4])
                prev = (qs_c, ks_c, vt)
            # final retention wave (t = T)
            qs_p, ks_p, vt_p = prev
            sd = {}
            for hp in range(H // 2):
                ret_scores(hp, qs_p, ks_p, sd)
                ret_out(hp, sd, vt_p, T)

        # =========== stage B: proj + MLP + output ===========
        with tc.tile_pool(name="mwork", bufs=1) as mwork, \
             tc.tile_pool(name="xtin_pool", bufs=1) as xtin_pool, \
             tc.tile_pool(name="psM", bufs=1, space="PSUM") as psM:
            cp = {}
            c1 = {}
            c2 = {}
            x2_all = {}
            ht_all = {}

            def fc2_wave(t):
                htile = ht_all.pop(t)
                x2 = x2_all.pop(t)
                last = (t == T)
                specs = []
                sts = []
                for ot in range(CT):
                    ps = psM.tile([128, N], f32, name="psf2", tag="psf2",
                                  bufs=2)
                    cin = c2.get(ot)
                    for kt in range(HT):
                        nc.tensor.matmul(
                            ps[:], w2t[:, kt, :, ot * 128:(ot + 1) * 128],
                            _dr2(htile[kt][:]), start=(kt == 0),
                            stop=(kt == HT - 1) and cin is None, perf_mode=DR)
                    if cin is not None:
                        nc.tensor.matmul(ps[:], ids[t], cin[:],
                                         start=False, stop=True)
                    A = work.tile([128, 512], f32, name="lifA", tag="lifA",
                                  bufs=6)
                    ACT.activation(A[:], ps[:], Act.Identity,
                                   bias=bias_sb["b2"][:, t - 1, ot:ot + 1],
                                   scale=float(2.0 ** (t - 1)))
                    st = mwork.tile([128, N], fp16, name="spk_m", tag="spk_m",
                                    bufs=2)
                    sts.append(st)
                    specs.append((A[:], thr_sb["b2"][:, t - 1, ot:ot + 1], st,
                                  mwork, f"c2_{ot}" if t < T else None, c2, ot))
                emit_spikes(specs)
                emit_carries(specs)
                outb = mwork.tile([128, CT, N], fp16, name="outb", tag="outb",
                                  bufs=1)
                for ot in range(CT):
                    DVE.tensor_tensor(outb[:, ot, :], x2[ot], sts[ot][:],
                                      Alu.add)
                    if last:
                        nc.sync.dma_start(out_e[t - 1, ot], outb[:, ot, :])
                if not last:
                    ACT.dma_start(
                        out_e[t - 1].rearrange("ct p n -> p ct n"), outb[:])

            for t in range(1, T + 1):
                xin = xtin_pool.tile([128, CT, N], f32r, name="xtin",
                                     tag="xtin", bufs=2)
                for kt in range(CT):
                    nc.sync.dma_start(xin[:, kt, :], xb[t - 1, kt])
                # proj: DR residual weights, fp8 retention-spike ifmap
                x2 = {}
                specs = []
                stps = []
                for ot in range(CT):
                    ps = psM.tile([128, N], f32, name="psp", tag="psp", bufs=2)
                    cin = cp.get(ot)
                    for kt in range(CT):
                        nc.tensor.matmul(
                            ps[:], pwt[:, kt, :, ot * 128:(ot + 1) * 128],
                            _dr2(os_[t - 1, kt][:]), start=(kt == 0),
                            stop=(kt == CT - 1) and cin is None, perf_mode=DR)
                    if cin is not None:
                        nc.tensor.matmul(ps[:], ids[t], cin[:],
                                         start=False, stop=True)
                    A = work.tile([128, 512], f32, name="lifA", tag="lifA",
                                  bufs=6)
                    ACT.activation(A[:], ps[:], Act.Identity,
                                   bias=bias_sb["pb"][:, t - 1, ot:ot + 1],
                                   scale=float(2.0 ** (t - 1)))
                    stp = mwork.tile([128, N], fp16, name="spk_p",
                                     tag="spk_p", bufs=2)
                    stps.append(stp)
                    specs.append((A[:], thr_sb["pb"][:, t - 1, ot:ot + 1],
                                  stp, mwork, f"cp_{ot}" if t < T else None,
                                  cp, ot))
                emit_spikes(specs)
                emit_carries(specs)
                x2b = mwork.tile([128, CT, N], f32r, name="x2t", tag="x2t",
                                 bufs=2)
                for ot in range(CT):
                    DVE.tensor_tensor(x2b[:, ot, :], xin[:, ot, :],
                                      stps[ot][:], Alu.add)
                    x2[ot] = x2b[:, ot, :]
                x2_all[t] = x2
                if t > 1:
                    fc2_wave(t - 1)
                htile = {}
                specs = []
                for ot in range(HT):
                    ps = psM.tile([128, N], f32, name="psf1", tag="psf1",
                                  bufs=4)
                    cin = c1.get(ot)
                    for kt in range(CT):
                        nc.tensor.matmul(
                            ps[:], w1t[:, kt, ot * 128:(ot + 1) * 128],
                            x2[kt], start=(kt == 0),
                            stop=(kt == CT - 1) and cin is None)
                    if cin is not None:
                        nc.tensor.matmul(ps[:], ids[t], cin[:],
                                         start=False, stop=True)
                    A = work.tile([128, 512], f32, name="lifA", tag="lifA",
                                  bufs=6)
                    ACT.activation(A[:], ps[:], Act.Identity,
                                   bias=bias_sb["b1"][:, t - 1, ot:ot + 1],
                                   scale=float(2.0 ** (t - 1)))
                    st = mwork.tile([128, N], fp8, name="spk_h", tag="spk_h",
                                    bufs=HT)
                    htile[ot] = st
                    specs.append((A[:], float(2.0 ** t), st, mwork,
                                  f"c1_{ot}" if t < T else None, c1, ot))
                    if ot % 4 == 3:  # flush quarters to bound A-tile usage
                        emit_spikes(specs)
                        emit_carries(specs)
                        specs = []
                ht_all[t] = htile
            fc2_wave(T)

    nc.finalize()
    return nc


def _host_prep(inputs):
    def fold(w, b, bn):
        g, bb, m, v = [bn[i].astype(np.float64) for i in range(4)]
        A = g / np.sqrt(v + EPS)
        W = w.astype(np.float64) * A[:, None]
        bias = (b.astype(np.float64) - m) * A + bb
        return W, bias

    def col_layout(WT, dt):
        # [ci, co] -> [128, ci//128, co]
        ci, co = WT.shape
        return np.ascontiguousarray(
            WT.reshape(ci // 128, 128, co).transpose(1, 0, 2).astype(dt))

    def bias_layout(vals):
        # per-t scaled rows: [co] -> [128, T, co//128]
        co = vals.shape[0]
        arr = np.stack([(vals * (2.0 ** t)).reshape(co // 128, 128).T
                        for t in range(T)], axis=1)
        return np.ascontiguousarray(arr.astype(np.float32))

    feed = {}
    wstack = {}
    biases = np.zeros((128, T, 32), np.float32)
    thrs = np.zeros((128, T, 8), np.float32)
    bslot = {"qw": 0, "kw": 4, "pw": 8, "w2": 12, "w1": 16}
    tslot = {"pw": 0, "w2": 4}
    for nm, bkey, bnkey in [("qw", "qb", "qbn"), ("kw", "kb", "kbn"),
                            ("vw", "vb", "vbn"), ("pw", "pb", "pbn"),
                            ("w1", "b1", "bn1"), ("w2", "b2", "bn2")]:
        W, bias = fold(inputs[nm], inputs[bkey], inputs[bnkey])
        if nm == "vw":
            feed["wqkv_v"] = col_layout(W.T, np.float32)
            vrow = np.zeros((1, C + 128), np.float32)
            vrow[0, :C] = bias.astype(np.float32)
            vrow[0, C:] = 1.0
            feed["vrow"] = vrow
            continue
        if nm in ("pw", "w2"):
            # per-channel scale, e4m3 + residual packing
            s = 1.0 / np.sqrt(np.mean(W ** 2, axis=1))
            Ws = (W * s[:, None]).astype(np.float32)
            WT = Ws.T  # [ci, co]
            ci, co = WT.shape
            Q = WT.astype(E4)
            R = (WT - Q.astype(np.float32)).astype(E4)
            pair = np.stack([Q, R], axis=1)  # [ci, 2, co]
            feed[nm] = np.ascontiguousarray(
                pair.reshape(ci // 128, 128, 2, co)
                .transpose(1, 0, 2, 3).astype(E4))
            # extract bias = s_c*b*2^(t-1); spike thr = s_c*2^t = (2 s_c)*2^(t-1)
            sb = (bias * s).astype(np.float64)
            biases[:, :, bslot[nm]:bslot[nm] + co // 128] = bias_layout(sb)
            thrs[:, :, tslot[nm]:tslot[nm] + co // 128] = \
                bias_layout(2.0 * s.astype(np.float64))
            continue
        wl = col_layout(W.T, np.float32)
        if nm in ("qw", "kw"):
            wstack[nm] = wl
        else:
            feed[nm] = wl
        co = bias.shape[0]
        biases[:, :, bslot[nm]:bslot[nm] + co // 128] = bias_layout(bias)
    feed["biases"] = biases
    feed["thrs"] = thrs
    feed["wqkv"] = np.ascontiguousarray(
        np.concatenate([wstack["qw"], wstack["kw"], feed.pop("wqkv_v")],
                       axis=1))

    gamma = 1.0 - 2.0 ** (-5.0 - np.arange(H, dtype=np.float64))
    idx = np.arange(N, dtype=np.float64)
    dist = np.abs(idx[:, None] - idx[None, :])
    scale = (C // H) ** -0.5
    dm = np.empty((H, 128, NT, N), np.float16)
    for h in range(H):
        dm[h] = ((gamma[h] ** dist) * scale * 0.5).reshape(
            NT, 128, N).transpose(1, 0, 2).astype(np.float16)
    feed["dmat"] = dm

    idm = np.zeros((128, 3, 128), np.float32)
    for i, sc in enumerate((0.5, 0.25, 0.125)):
        idm[:, i, :] = sc * np.eye(128, dtype=np.float32)
    feed["idm"] = idm
    return feed


def kernel(**inputs):
    if "nc" not in _CACHE:
        _CACHE["nc"] = _build()
    nc = _CACHE["nc"]
    feed = _host_prep(inputs)
    x = inputs["x"]
    in_maps = []
    for b in range(B):
        m = dict(feed)
        xt = x[:, b].transpose(0, 2, 1).reshape(T, CT, 128, N)
        m["xb"] = np.ascontiguousarray(xt)
        in_maps.append(m)
    res = None
    last_err = None
    for _attempt in range(3):
        try:
            res = run_bass_kernel_spmd(nc, in_maps, list(range(B)))
            break
        except Exception as e:  # transient NRT device wedges recover on retry
            last_err = e
    if res is None:
        raise last_err
    out = np.empty((T, B, N, C), np.float32)
    for b in range(B):
        oT = res.results[b]["out"].reshape(T, C, N).astype(np.float32)
        out[:, b] = oT.transpose(0, 2, 1)
    return out
